# revision 1
# baseline (speedup 1.0000x reference)
"""Trainium2 Bass kernel: single-head causal attention, SPMD over 8 NeuronCores.

Problem: x [4, 2048, 1024] f32; Wq/Wk/Wv [1024, 64]; bq/bk/bv [64].
  q,k,v = x@W + b ; out = softmax(causal(q k^T / 8)) @ v  -> [4, 2048, 64]

Sharding (uniform SPMD structure on every core):
  core c -> batch b = c//2 ; query chunks (cA, cB) = (c%2, 3-c%2), 512 rows
  each (pairing an early with a late chunk balances causal work).  Every core
  computes K/V for its batch's full 2048 rows; collectives would cost more
  than the duplicated projection at this size.

Key layout trick: the k-axis is permuted PER CORE to chunk order
  [cA, 1-cA, 5-cB, cB], so the core's own query columns sit at the STATIC
  positions 0:512 and 1536:2048 of the K/V input -- Q projection needs no
  separate input tensor.  Causality is enforced by data-driven per-partition
  thresholds (thr) against a free-axis iota, which absorb the permutation;
  k-tiles 0..7 for the late slot are causally full for every core and skip
  masking entirely, and the early slot structurally uses only k-tiles 0..7.

  Projections produce Q^T/K^T/V^T [64, rows]; scores are computed transposed
  ([k_part, q_free]) so the attention-weight matrix feeds the AV matmul as
  the moving operand; V is re-transposed through 16 small PE transposes; a
  65th "ones" row on the V tiles makes the AV matmul accumulate the softmax
  denominator for free.  Score matmuls (K=64) are row-packed in pairs into
  disjoint PE row-groups via duplicated K^T/Q^T at partitions 64:127.

dtypes: fp16 SBUF operands (1 cycle/row on the PE; integers exact to 2048
  for the mask iota), fp32 PSUM accumulation, fp32 epilogue + output.
"""

import os
import sys

import numpy as np

if "/opt/trn_rl_repo" not in sys.path:
    sys.path.insert(0, "/opt/trn_rl_repo")

B, S, D, H = 4, 2048, 1024, 64
CH = 512          # query chunk width
QR = 2 * CH       # query rows per core
NKT = S // 128    # 16 k-tiles of 128
SLOT_KT = (8, 16)  # k-tiles consumed by slot A / slot B
SCALE = 1.0 / np.sqrt(H)

_CACHE = {}


def _build_nc():
    import concourse.bacc as bacc
    import concourse.mybir as mybir
    import concourse.tile as tile

    DT = mybir.dt.float16
    F32 = mybir.dt.float32
    Exp = mybir.ActivationFunctionType.Exp
    Copy = mybir.ActivationFunctionType.Copy
    ge = mybir.AluOpType.is_ge
    mult = mybir.AluOpType.mult
    add = mybir.AluOpType.add

    nc = bacc.Bacc("TRN2", target_bir_lowering=False, debug=False, num_devices=8)

    # xk: k-permuted x^T in 16 contiguous [128, 1024] chunks;
    # row block kt*2+h holds dmodel-tile kt, k-position half h.
    xk = nc.dram_tensor("xk", [16 * 128, 1024], DT, kind="ExternalInput")
    wkv = nc.dram_tensor("wkv", [8 * 128, 128], DT, kind="ExternalInput")
    wq = nc.dram_tensor("wq", [128, 8 * H], DT, kind="ExternalInput")
    bkv = nc.dram_tensor("bkv", [128, 1], F32, kind="ExternalInput")
    bq = nc.dram_tensor("bq", [H, 1], F32, kind="ExternalInput")
    qio = nc.dram_tensor("qio", [128, CH], DT, kind="ExternalInput")
    thr = nc.dram_tensor("thr", [128, 2 * NKT], F32, kind="ExternalInput")
    idv = nc.dram_tensor("idv", [128, H], DT, kind="ExternalInput")
    id16 = nc.dram_tensor("id16", [H + 1, H + 1], DT, kind="ExternalInput")
    out = nc.dram_tensor("out", [QR, H], F32, kind="ExternalOutput")

    with tile.TileContext(nc) as tc:
        with (
            tc.tile_pool(name="const", bufs=1) as cp,
            tc.tile_pool(name="work", bufs=8) as wp,
            tc.tile_pool(name="epi", bufs=4) as ep,
        ):
            # ---- head: the first matmul needs only wkv[0] + xk[0][0], so
            # those are the very first issues on their engines.
            issue4 = [nc.sync, nc.scalar, nc.gpsimd]
            wkv_sb = cp.tile([128, 8 * 128], DT, tag="wkv", name="wkv")
            xk_sb = [[None, None] for _ in range(8)]

            def _xk_tile(kt, h):
                t = cp.tile([128, 1024], DT, tag=f"xk{kt}_{h}",
                            name=f"xk{kt}_{h}")
                xk_sb[kt][h] = t
                return t, (kt * 2 + h) * 128

            t0, row0 = _xk_tile(0, 0)
            nc.sync.dma_start(t0[0:64, :], xk[row0:row0 + 64, :])
            nc.scalar.dma_start(t0[64:128, :], xk[row0 + 64:row0 + 128, :])
            nc.gpsimd.dma_start(wkv_sb[:, 0:128], wkv[0:128, :])
            for kt in range(1, 8):
                issue4[kt % 3].dma_start(
                    wkv_sb[:, kt * 128:(kt + 1) * 128],
                    wkv[kt * 128:(kt + 1) * 128, :])
            wq_sb = cp.tile([128, 8 * H], DT, tag="wq", name="wq")
            nc.gpsimd.dma_start(wq_sb[:], wq[:])
            bkv_sb = cp.tile([128, 1], F32, tag="bkv", name="bkv")
            nc.gpsimd.dma_start(bkv_sb[:], bkv[:])
            bq_sb = cp.tile([H, 1], F32, tag="bq", name="bq")
            nc.gpsimd.dma_start(bq_sb[:], bq[:])

            # remaining x chunks in consumption order; h=1 off scalar so the
            # ACT engine is free when the first exp ops arrive
            n_issued = 0
            for h in range(2):
                for kt in range(8):
                    if h == 0 and kt == 0:
                        continue
                    t, row = _xk_tile(kt, h)
                    engs = issue4 if h == 0 else [nc.sync, nc.gpsimd]
                    # kt 1-2 of half 0 gate the PE right after the first
                    # matmul group: quarter them across 4 queues each
                    nsplit = 4 if (h == 0 and kt <= 4) else 2
                    step = 128 // nsplit
                    for s in range(nsplit):
                        engs[n_issued % len(engs)].dma_start(
                            t[s * step:(s + 1) * step, :],
                            xk[row + s * step:row + (s + 1) * step, :])
                        n_issued += 1

            # late-use constants (mask iota/thr, identities)
            qio_sb = cp.tile([128, CH], DT, tag="qio", name="qio")
            nc.gpsimd.dma_start(qio_sb[:], qio[:])
            thr_sb = cp.tile([128, 2 * NKT], F32, tag="thr", name="thr")
            nc.gpsimd.dma_start(thr_sb[:], thr[:])
            idv_sb = cp.tile([128, H], DT, tag="idv", name="idv")
            nc.scalar.dma_start(idv_sb[:], idv[:])
            id16_sb = cp.tile([H + 1, H + 1], DT, tag="id16", name="id16")
            nc.gpsimd.dma_start(id16_sb[:], id16[:])

            kvT_sb = cp.tile([128, S], DT, tag="kvT", name="kvT")  # 0:64 K^T, 64:128 V^T
            qT_sb = cp.tile([H, QR], DT, tag="qT", name="qT")      # A cols 0:512, B 512:1024
            v_sb = cp.tile([128, NKT * (H + 1)], DT, tag="v", name="v")
            # duplicates at partitions 64:127 for row-packed score pairs
            ktd_sb = cp.tile([128, S], DT, tag="ktd", name="ktd")
            qTd_sb = cp.tile([128, QR], DT, tag="qTd", name="qTd")
            vtd_sb = cp.tile([64, S], DT, tag="vtd", name="vtd")

            # ---- projections in two 3-bank PSUM phase scopes so the score
            # pool can allocate after phase h0 and slot-A attention overlaps
            # phase h1 (banks: h1 3 + score 4 = 7; then score 4 + av 4 = 8)
            sp = None
            for h in range(2):
                with tc.tile_pool(name=f"proj_ps{h}", bufs=1,
                                  space="PSUM") as pp:
                    kv_ps = [pp.tile([128, 512], F32, tag=f"kvps{h}{s}",
                                     name=f"kvps{h}{s}") for s in range(2)]
                    q_ps = pp.tile([H, 512], F32, tag=f"qps{h}",
                                   name=f"qps{h}")
                    # q columns: slot A = positions 0:512 (in half 0),
                    # slot B = positions 1536:2048 (in half 1)
                    qcol = slice(0, 512) if h == 0 else slice(512, 1024)
                    for kt in range(8):
                        for sub in range(2):
                            nc.tensor.matmul(
                                kv_ps[sub][:],
                                wkv_sb[:, kt * 128:(kt + 1) * 128],
                                xk_sb[kt][h][:, sub * 512:(sub + 1) * 512],
                                start=(kt == 0), stop=(kt == 7),
                            )
                        nc.tensor.matmul(
                            q_ps[:],
                            wq_sb[:, kt * H:(kt + 1) * H],
                            xk_sb[kt][h][:, qcol],
                            start=(kt == 0), stop=(kt == 7),
                        )
                    for sub in range(2):
                        nb = 2 * h + sub
                        nc.vector.tensor_scalar(
                            kvT_sb[:, nb * 512:(nb + 1) * 512], kv_ps[sub][:],
                            bkv_sb[:], None, add)
                        # K^T duplicate rows 64:127 (small SBUF->SBUF DMA,
                        # off the critical path)
                        nc.sync.dma_start(
                            ktd_sb[H:128, nb * 512:(nb + 1) * 512],
                            kvT_sb[0:H, nb * 512:(nb + 1) * 512])
                        nc.gpsimd.dma_start(
                            vtd_sb[:, nb * 512:(nb + 1) * 512],
                            kvT_sb[H:128, nb * 512:(nb + 1) * 512])
                    nc.vector.tensor_scalar(
                        qT_sb[:, h * 512:(h + 1) * 512], q_ps[:],
                        bq_sb[:], None, add)
                    nc.scalar.dma_start(
                        qTd_sb[H:128, h * 512:(h + 1) * 512],
                        qT_sb[:, h * 512:(h + 1) * 512])
                if h == 0:
                    sp = tc.alloc_tile_pool(name="score_ps", bufs=4,
                                            space="PSUM")
                    avpA = tc.alloc_tile_pool(name="avA_ps", bufs=1,
                                              space="PSUM")
            avpB = tc.alloc_tile_pool(name="avB_ps", bufs=1, space="PSUM")

            # ---- V^T -> V tiles (+ ones column), transposes row-packed ----
            nc.vector.memset(v_sb[:], 1.0)
            for pr in range(NKT // 2):
                k0, k1 = 2 * pr, 2 * pr + 1
                t0 = sp.tile([128, H], DT, tag="score", name="vtr0")
                nc.tensor.transpose(
                    t0[:], vtd_sb[:, k0 * 128:(k0 + 1) * 128],
                    idv_sb[0:H, :], tile_position=(0, 0))
                t1 = sp.tile([128, H], DT, tag="score", name="vtr1")
                nc.tensor.transpose(
                    t1[:], kvT_sb[64:128, k1 * 128:(k1 + 1) * 128],
                    idv_sb[64:64 + H, :], tile_position=(64, 0))
                nc.vector.tensor_copy(
                    v_sb[:, k0 * (H + 1):k0 * (H + 1) + H], t0[:])
                nc.vector.tensor_copy(
                    v_sb[:, k1 * (H + 1):k1 * (H + 1) + H], t1[:])

            # ---- attention per slot (score pairs row-packed) ----
            for slot in range(2):
                nkt = SLOT_KT[slot]
                if slot == 0:
                    av_u = avpA.tile([H + 1, 512], F32, tag="avA", name="avA")
                else:
                    av_e = avpB.tile([H + 1, 512], F32, tag="avE", name="avE")
                    av_o = avpB.tile([H + 1, 512], F32, tag="avO", name="avO")
                # slot B: masked tiles (8..15) first so the cheap unmasked
                # tail keeps the final AV dependency chain short
                kts = list(range(8, 16)) + list(range(8)) if slot == 1 else list(range(nkt))
                for ki in range(0, nkt, 2):
                    kt0, kt1 = kts[ki], kts[ki + 1]
                    s0 = sp.tile([128, 512], F32, tag="score", name="score0")
                    nc.tensor.matmul(
                        s0[:],
                        kvT_sb[0:H, kt0 * 128:(kt0 + 1) * 128],
                        qT_sb[:, slot * 512:(slot + 1) * 512],
                        start=True, stop=True, tile_position=(0, 0),
                    )
                    s1 = sp.tile([128, 512], F32, tag="score", name="score1")
                    nc.tensor.matmul(
                        s1[:],
                        ktd_sb[H:128, kt1 * 128:(kt1 + 1) * 128],
                        qTd_sb[H:128, slot * 512:(slot + 1) * 512],
                        start=True, stop=True, tile_position=(64, 0),
                    )
                    w_pair = []
                    for s_ps in (s0, s1):
                        w_sb = wp.tile([128, 512], DT, tag="wexp", name="wexp")
                        nc.scalar.activation(w_sb[:], s_ps[:], Exp,
                                             scale=float(SCALE))
                        w_pair.append(w_sb)
                    wav_pair = []
                    for kt, w_sb in zip((kt0, kt1), w_pair):
                        if slot == 1 and kt < 8:
                            wav_pair.append(w_sb)
                            continue
                        idx = slot * NKT + kt
                        m_sb = wp.tile([128, 512], DT, tag="msk", name="msk")
                        nc.vector.tensor_scalar(
                            m_sb[:], qio_sb[:], thr_sb[:, idx:idx + 1], None, ge)
                        wm_sb = wp.tile([128, 512], DT, tag="wm", name="wm")
                        nc.vector.tensor_tensor(
                            wm_sb[:], w_sb[:], m_sb[:], mult)
                        wav_pair.append(wm_sb)
                    for j, (kt, w_av) in enumerate(zip((kt0, kt1), wav_pair)):
                        vs = slice(kt * (H + 1), (kt + 1) * (H + 1))
                        if slot == 0:
                            nc.tensor.matmul(
                                av_u[:], v_sb[:, vs], w_av[:],
                                start=(ki + j == 0),
                                stop=(ki + j == nkt - 1),
                            )
                        else:
                            nc.tensor.matmul(
                                av_e[:], v_sb[0:H, vs], w_av[0:H, :],
                                start=(ki + j == 0), stop=(ki + j == nkt - 1),
                                tile_position=(0, 0),
                            )
                            nc.tensor.matmul(
                                av_o[:], v_sb[H:128, vs], w_av[H:128, :],
                                start=(ki + j == 0), stop=(ki + j == nkt - 1),
                                tile_position=(64, 0),
                            )
                # epilogue: sum AV halves (ACT copy + DVE add, fp16),
                # transpose to [128, 65], normalize in f32
                oav_sb = ep.tile([H + 1, 512], DT, tag="oav16", name="oav")
                if slot == 0:
                    for j in range(4):
                        js = slice(j * 128, (j + 1) * 128)
                        nc.scalar.activation(oav_sb[:, js], av_u[:, js], Copy)
                else:
                    oc_sb = ep.tile([H + 1, 512], F32, tag="oav", name="oavc")
                    for j in range(4):
                        js = slice(j * 128, (j + 1) * 128)
                        nc.scalar.activation(oc_sb[:, js], av_e[:, js], Copy)
                        nc.vector.tensor_tensor(
                            oav_sb[:, js], oc_sb[:, js], av_o[:, js], add)
                for j in range(4):
                    tr_ps = sp.tile([128, H + 1], DT, tag="score", name="otr")
                    nc.tensor.transpose(
                        tr_ps[:],
                        oav_sb[:, j * 128:(j + 1) * 128],
                        id16_sb[0:H + 1, 0:H + 1],
                    )
                    r_sb = ep.tile([128, 1], F32, tag="recip", name="recip")
                    nc.vector.reciprocal(r_sb[:], tr_ps[:, H:H + 1])
                    o_sb = ep.tile([128, H], F32, tag="osb", name="osb")
                    nc.vector.tensor_scalar_mul(o_sb[:], tr_ps[:, 0:H], r_sb[:])
                    row = slot * CH + j * 128
                    # sync/scalar only: a gpsimd-issued store would hold up
                    # gpsimd's end-of-kernel queue drain by ~3us
                    (nc.sync if j % 2 == 0 else nc.scalar).dma_start(
                        out[row:row + 128, :], o_sb[:])

            for pool in (avpB, avpA, sp):
                pool.release()

    nc.compile()
    return nc


def _host_inputs(x, Wq, bq, Wk, bk, Wv, bv):
    """Build the 8 per-core input maps (all SBUF-layout, fp16/f32)."""
    f16 = np.float16
    Wkv = np.concatenate([Wk, Wv], axis=1)          # [D, 128]
    wkv_np = np.ascontiguousarray(Wkv).astype(f16).reshape(8 * 128, 128)
    wq_np = np.zeros((128, 8 * H), dtype=f16)
    for kt in range(8):
        wq_np[:, kt * H:(kt + 1) * H] = Wq[kt * 128:(kt + 1) * 128, :]
    bkv_np = np.concatenate([bk, bv]).reshape(128, 1).astype(np.float32)
    bq_np = bq.reshape(H, 1).astype(np.float32)
    qio_np = np.broadcast_to(np.arange(CH, dtype=f16), (128, CH)).copy()
    idv_np = np.concatenate([np.eye(H), np.eye(H)], axis=0).astype(f16)
    id16_np = np.eye(H + 1, dtype=f16)

    in_maps = []
    for c in range(8):
        b = c // 2
        cA, cB = c % 2, 3 - c % 2
        perm = (cA, 1 - cA, 5 - cB, cB)        # chunk order along k
        xTp = np.concatenate(
            [x[b, p * CH:(p + 1) * CH].T for p in perm], axis=1)  # [D, S]
        xTp = xTp.astype(f16)
        xk_np = np.zeros((16 * 128, 1024), dtype=f16)
        for kt in range(8):
            for h in range(2):
                xk_np[(kt * 2 + h) * 128:(kt * 2 + h + 1) * 128] = \
                    xTp[kt * 128:(kt + 1) * 128, h * 1024:(h + 1) * 1024]
        # k_global of permuted position p: perm[p//512]*512 + p%512
        pos = np.arange(S)
        kg = np.array(perm)[pos // CH] * CH + pos % CH
        thr_np = np.zeros((128, 2 * NKT), dtype=np.float32)
        p = np.arange(128)
        for slot, ck in enumerate((cA, cB)):
            for kt in range(NKT):
                thr_np[:, slot * NKT + kt] = kg[kt * 128 + p] - ck * CH
        in_maps.append({
            "xk": xk_np, "wkv": wkv_np, "wq": wq_np,
            "bkv": bkv_np, "bq": bq_np, "qio": qio_np, "thr": thr_np,
            "idv": idv_np, "id16": id16_np,
        })
    return in_maps


def _gather(results, dtype):
    y = np.zeros((B, S, H), dtype=dtype)
    for c in range(8):
        b = c // 2
        cA, cB = c % 2, 3 - c % 2
        o = results[c]["out"]
        y[b, cA * CH:(cA + 1) * CH] = o[:CH]
        y[b, cB * CH:(cB + 1) * CH] = o[CH:]
    return y


def get_nc():
    if "nc" not in _CACHE:
        _CACHE["nc"] = _build_nc()
    return _CACHE["nc"]


def kernel(x, Wq, bq, Wk, bk, Wv, bv, _trace=False, _trace_kwargs=None):
    from concourse.bass_utils import run_bass_kernel_spmd

    x = np.asarray(x, dtype=np.float32)
    Wq, bq = np.asarray(Wq, np.float32), np.asarray(bq, np.float32)
    Wk, bk = np.asarray(Wk, np.float32), np.asarray(bk, np.float32)
    Wv, bv = np.asarray(Wv, np.float32), np.asarray(bv, np.float32)

    nc = get_nc()
    in_maps = _host_inputs(x, Wq, bq, Wk, bk, Wv, bv)
    res = run_bass_kernel_spmd(
        nc, in_maps, core_ids=list(range(8)),
        trace=_trace, **(_trace_kwargs or {}))
    _CACHE["last_result"] = res
    return _gather(res.results, x.dtype)



# revision 9
# speedup vs baseline: 1.0350x; 1.0350x over previous
"""Trainium2 Bass kernel: single-head causal attention, SPMD over 8 NeuronCores.

Problem: x [4, 2048, 1024] f32; Wq/Wk/Wv [1024, 64]; bq/bk/bv [64].
  q,k,v = x@W + b ; out = softmax(causal(q k^T / 8)) @ v  -> [4, 2048, 64]

Sharding (uniform SPMD structure on every core):
  core c -> batch b = c//2 ; query chunks (cA, cB) = (c%2, 3-c%2), 512 rows
  each (pairing an early with a late chunk balances causal work).  Every core
  computes K/V for its batch's full 2048 rows.

Key layout: the k-axis is permuted PER CORE to chunk order
  [cA, 1-cA, 5-cB, cB], so the core's own query columns sit at the STATIC
  positions 0:512 and 1536:2048 of the K/V input.  With that permutation the
  24 (slot, k-tile) score tiles fall into three static classes:
    diag        A:0-3,  B:12-15  -- element-wise causal mask
    conditional A:4-7,  B:8-11   -- fully dead or fully allowed per core
    full        B:0-7            -- causally full for every core
  Masking is folded into the exp: diag tiles add a precomputed 0/+512 bias
  tile then exp(s*scale - 64); conditional tiles add a per-core 0/-400 bias
  column; dead tiles underflow to exactly 0 in fp16, so no mask multiplies
  and the 65th "ones" V row still accumulates the correct denominator.

  Projections produce Q^T/K^T/V^T [64, rows]; scores are computed transposed
  ([k_part, q_free]) so the weight matrix feeds the AV matmul as the moving
  operand; V is re-transposed through 16 small PE transposes.  Both slots
  accumulate AV in a single K=128 PSUM accumulator.  Slot-A attention and
  the V transposes are interleaved into the h1 projection stream so the PE
  never idles (keeps the tensor clock in its high p-state).

  Input DMAs are coalesced: xk is repacked host-side to [128, 16*1024] in
  consumption order, loaded by 8 large descriptors spread over the
  sync/scalar/vector hardware queues; tiny constants ride the gpsimd queue.

dtypes: fp16 SBUF operands, fp32 PSUM accumulation, fp32 epilogue + output.
"""

import os
import sys

import numpy as np

if "/opt/trn_rl_repo" not in sys.path:
    sys.path.insert(0, "/opt/trn_rl_repo")

B, S, D, H = 4, 2048, 1024, 64
CH = 512          # query chunk width
QR = 2 * CH       # query rows per core
NKT = S // 128    # 16 k-tiles of 128
SCALE = 1.0 / np.sqrt(H)
MBIG = 512.0      # diag mask additive bias (exactly representable, *SCALE=64)
CBIG = 400.0      # conditional (dead-tile) bias magnitude

_CACHE = {}


def _build_nc():
    import concourse.bacc as bacc
    import concourse.mybir as mybir
    import concourse.tile as tile

    DT = mybir.dt.float16
    F32 = mybir.dt.float32
    Exp = mybir.ActivationFunctionType.Exp
    Copy = mybir.ActivationFunctionType.Copy
    ge = mybir.AluOpType.is_ge
    mult = mybir.AluOpType.mult
    add = mybir.AluOpType.add

    nc = bacc.Bacc("TRN2", target_bir_lowering=False, debug=False, num_devices=8)

    # xk: k-permuted x^T, [128, 16*1024]; column block h*8+kt holds
    # dmodel-tile kt of k-half h (consumption order, so one coalesced
    # stream fills it front to back).
    xk = nc.dram_tensor("xk", [128, 16 * 1024], DT, kind="ExternalInput")
    wkv = nc.dram_tensor("wkv", [128, 8 * 128], DT, kind="ExternalInput")
    wq = nc.dram_tensor("wq", [128, 8 * H], DT, kind="ExternalInput")
    bkv = nc.dram_tensor("bkv", [128, 1], F32, kind="ExternalInput")
    bq = nc.dram_tensor("bq", [H, 1], F32, kind="ExternalInput")
    qio = nc.dram_tensor("qio", [128, CH], DT, kind="ExternalInput")
    thrd = nc.dram_tensor("thrd", [128, 8], F32, kind="ExternalInput")
    thrb = nc.dram_tensor("thrb", [128, 8], F32, kind="ExternalInput")
    idv = nc.dram_tensor("idv", [128, H], DT, kind="ExternalInput")
    id16 = nc.dram_tensor("id16", [H + 1, H + 1], DT, kind="ExternalInput")
    out = nc.dram_tensor("out", [QR, H], F32, kind="ExternalOutput")

    def xcol(kt, h, sub=0, n=512):
        base = (h * 8 + kt) * 1024 + sub * 512
        return slice(base, base + n)

    with tile.TileContext(nc) as tc:
        with (
            tc.tile_pool(name="const", bufs=1) as cp,
            tc.tile_pool(name="work", bufs=8) as wp,
            tc.tile_pool(name="epi", bufs=4) as ep,
        ):
            # ---- coalesced input DMAs, consumption order ----
            wkv_sb = cp.tile([128, 8 * 128], DT, tag="wkv", name="wkv")
            xk_sb = cp.tile([128, 16 * 1024], DT, tag="xk", name="xk")
            wq_sb = cp.tile([128, 8 * H], DT, tag="wq", name="wq")

            nc.sync.dma_start(wkv_sb[:], wkv[:])
            nc.scalar.dma_start(xk_sb[:, 0:1024], xk[:, 0:1024])         # h0 kt0
            nc.sync.dma_start(xk_sb[:, 1024:2048], xk[:, 1024:2048])     # h0 kt1
            nc.scalar.dma_start(wq_sb[:], wq[:])
            nc.sync.dma_start(xk_sb[:, 2048:4096], xk[:, 2048:4096])     # h0 kt2-3
            nc.scalar.dma_start(xk_sb[:, 4096:6144], xk[:, 4096:6144])   # h0 kt4-5
            nc.sync.dma_start(xk_sb[:, 6144:8192], xk[:, 6144:8192])     # h0 kt6-7
            nc.scalar.dma_start(xk_sb[:, 8192:11264], xk[:, 8192:11264])  # h1 kt0-2
            nc.sync.dma_start(xk_sb[:, 11264:13312], xk[:, 11264:13312])  # h1 kt3-4
            nc.scalar.dma_start(xk_sb[:, 13312:16384], xk[:, 13312:16384])  # h1 kt5-7

            # tiny constants on the gpsimd (software) queue, needed-first order
            bkv_sb = cp.tile([128, 1], F32, tag="bkv", name="bkv")
            nc.gpsimd.dma_start(bkv_sb[:], bkv[:])
            bq_sb = cp.tile([H, 1], F32, tag="bq", name="bq")
            nc.gpsimd.dma_start(bq_sb[:], bq[:])
            qio_sb = cp.tile([128, CH], DT, tag="qio", name="qio")
            nc.gpsimd.dma_start(qio_sb[:], qio[:])
            thrd_sb = cp.tile([128, 8], F32, tag="thrd", name="thrd")
            nc.gpsimd.dma_start(thrd_sb[:], thrd[:])
            idv_sb = cp.tile([128, H], DT, tag="idv", name="idv")
            nc.gpsimd.dma_start(idv_sb[:], idv[:])
            thrb_sb = cp.tile([128, 8], F32, tag="thrb", name="thrb")
            nc.gpsimd.dma_start(thrb_sb[:], thrb[:])
            id16_sb = cp.tile([H + 1, H + 1], DT, tag="id16", name="id16")
            nc.gpsimd.dma_start(id16_sb[:], id16[:])

            nb_sb = cp.tile([128, 1], F32, tag="nb", name="nb")
            nc.vector.memset(nb_sb[:], float(-MBIG * SCALE))
            kvT_sb = cp.tile([128, S], DT, tag="kvT", name="kvT")  # 0:64 K^T, 64:128 V^T
            qT_sb = cp.tile([H, QR], DT, tag="qT", name="qT")      # A cols 0:512, B 512:1024
            v_sb = cp.tile([128, NKT * (H + 1)], DT, tag="v", name="v")
            mb_sb = cp.tile([128, 8 * CH], DT, tag="mb", name="mb")  # diag 0/+512 bias

            # ---- projections in two 3-bank PSUM phase scopes ----
            def proj_group(h, kt, kv_ps, q_ps):
                for sub in range(2):
                    nc.tensor.matmul(
                        kv_ps[sub][:],
                        wkv_sb[:, kt * 128:(kt + 1) * 128],
                        xk_sb[:, xcol(kt, h, sub)],
                        start=(kt == 0), stop=(kt == 7),
                    )
                # q columns: slot A = sub 0 of h0, slot B = sub 1 of h1
                nc.tensor.matmul(
                    q_ps[:],
                    wq_sb[:, kt * H:(kt + 1) * H],
                    xk_sb[:, xcol(kt, h, sub=h)],
                    start=(kt == 0), stop=(kt == 7),
                )

            def proj_epilogue(h, kv_ps, q_ps):
                for sub in range(2):
                    nb = 2 * h + sub
                    nc.vector.tensor_scalar(
                        kvT_sb[:, nb * 512:(nb + 1) * 512], kv_ps[sub][:],
                        bkv_sb[:], None, add)
                nc.vector.tensor_scalar(
                    qT_sb[:, h * 512:(h + 1) * 512], q_ps[:],
                    bq_sb[:], None, add)

            # ---- phase h0 ----
            pp0 = tc.alloc_tile_pool(name="proj_ps0", bufs=1, space="PSUM")
            kv_ps0 = [pp0.tile([128, 512], F32, tag=f"kvps0{s}",
                               name=f"kvps0{s}") for s in range(2)]
            q_ps0 = pp0.tile([H, 512], F32, tag="qps0", name="qps0")
            for kt in range(8):
                proj_group(0, kt, kv_ps0, q_ps0)
            proj_epilogue(0, kv_ps0, q_ps0)
            pp0.release()

            sp = tc.alloc_tile_pool(name="score_ps", bufs=4, space="PSUM")
            avpA = tc.alloc_tile_pool(name="avA_ps", bufs=1, space="PSUM")
            av_a = avpA.tile([H + 1, 512], F32, tag="avA", name="avA")

            # diag mask-bias tiles (A tiles 0..3 -> cols 0..3, B tiles
            # 12..15 -> cols 4..7): 0 / +512
            for j in range(8):
                nc.vector.tensor_scalar(
                    mb_sb[:, j * CH:(j + 1) * CH], qio_sb[:],
                    thrd_sb[:, j:j + 1], MBIG, ge, mult)

            def vtr(kt):
                t = sp.tile([128, H], DT, tag="score", name=f"vtr{kt}")
                nc.tensor.transpose(
                    t[:], kvT_sb[H:128, kt * 128:(kt + 1) * 128],
                    idv_sb[H:128, :], tile_position=(H, 0))
                nc.vector.tensor_copy(
                    v_sb[:, kt * (H + 1):kt * (H + 1) + H], t[:])

            nc.vector.memset(v_sb[:], 1.0)
            for kt in range(8):
                vtr(kt)

            # ---- slot helpers ----
            def score(slot, kt):
                s_ps = sp.tile([128, 512], F32, tag="score", name=f"s{slot}{kt}")
                nc.tensor.matmul(
                    s_ps[:],
                    kvT_sb[0:H, kt * 128:(kt + 1) * 128],
                    qT_sb[:, slot * 512:(slot + 1) * 512],
                    start=True, stop=True, tile_position=(0, 0),
                )
                return s_ps

            def wexp(slot, kt, s_ps):
                # tile class: diag (elementwise bias tile), cond (per-core
                # bias column), full (direct from PSUM)
                w_sb = wp.tile([128, 512], DT, tag="wexp", name="wexp")
                if slot == 0 and kt < 4:          # A diag
                    s1 = wp.tile([128, 512], F32, tag="s1", name="s1")
                    nc.vector.tensor_tensor(
                        s1[:], s_ps[:], mb_sb[:, kt * CH:(kt + 1) * CH], add)
                    nc.scalar.activation(w_sb[:], s1[:], Exp,
                                         bias=nb_sb[:],
                                         scale=float(SCALE))
                elif slot == 0 and kt >= 4:       # A conditional
                    s1 = wp.tile([128, 512], F32, tag="s1", name="s1")
                    nc.vector.tensor_scalar(
                        s1[:], s_ps[:], thrb_sb[:, kt - 4:kt - 3], None, add)
                    nc.scalar.activation(w_sb[:], s1[:], Exp, scale=float(SCALE))
                elif slot == 1 and 8 <= kt < 12:  # B conditional
                    s1 = wp.tile([128, 512], F32, tag="s1", name="s1")
                    nc.vector.tensor_scalar(
                        s1[:], s_ps[:], thrb_sb[:, kt - 4:kt - 3], None, add)
                    nc.scalar.activation(w_sb[:], s1[:], Exp, scale=float(SCALE))
                elif slot == 1 and kt >= 12:      # B diag
                    s1 = wp.tile([128, 512], F32, tag="s1", name="s1")
                    nc.vector.tensor_tensor(
                        s1[:], s_ps[:], mb_sb[:, (kt - 8) * CH:(kt - 7) * CH],
                        add)
                    nc.scalar.activation(w_sb[:], s1[:], Exp,
                                         bias=nb_sb[:],
                                         scale=float(SCALE))
                else:                             # B full (kt < 8)
                    nc.scalar.activation(w_sb[:], s_ps[:], Exp,
                                         scale=float(SCALE))
                return w_sb

            def av(acc, kt, w_sb, first, last):
                vs = slice(kt * (H + 1), (kt + 1) * (H + 1))
                nc.tensor.matmul(
                    acc[:], v_sb[:, vs], w_sb[:],
                    start=first, stop=last,
                )

            # ---- phase h1 interleaved with slot A attention ----
            pp1 = tc.alloc_tile_pool(name="proj_ps1", bufs=1, space="PSUM")
            kv_ps1 = [pp1.tile([128, 512], F32, tag=f"kvps1{s}",
                               name=f"kvps1{s}") for s in range(2)]
            q_ps1 = pp1.tile([H, 512], F32, tag="qps1", name="qps1")

            a_s = {}
            a_w = {}

            def asc_pair(p):
                for kt in (2 * p, 2 * p + 1):
                    a_s[kt] = score(0, kt)
                    a_w[kt] = wexp(0, kt, a_s[kt])

            def ava_pair(p):
                for kt in (2 * p, 2 * p + 1):
                    av(av_a, kt, a_w[kt], first=(kt == 0), last=(kt == 7))

            for kt in range(8):
                proj_group(1, kt, kv_ps1, q_ps1)
                # interleave slot-A work behind the projection groups
                if kt == 0:
                    asc_pair(0)
                elif kt == 1:
                    asc_pair(1)
                elif kt == 2:
                    ava_pair(0)
                elif kt == 3:
                    asc_pair(2)
                elif kt == 4:
                    ava_pair(1)
                elif kt == 5:
                    asc_pair(3)
                elif kt == 6:
                    ava_pair(2)
                else:
                    ava_pair(3)
            proj_epilogue(1, kv_ps1, q_ps1)
            pp1.release()

            avpB = tc.alloc_tile_pool(name="avB_ps", bufs=1, space="PSUM")
            av_b = avpB.tile([H + 1, 512], F32, tag="avB", name="avB")
            sp2 = tc.alloc_tile_pool(name="otr_ps", bufs=2, space="PSUM")

            def epilogue(slot, acc):
                oav_sb = ep.tile([H + 1, 512], DT, tag="oav16", name="oav")
                nc.scalar.activation(oav_sb[:], acc[:], Copy)
                for j in range(4):
                    tr_ps = sp2.tile([128, H + 1], DT, tag="otr", name="otr")
                    nc.tensor.transpose(
                        tr_ps[:],
                        oav_sb[:, j * 128:(j + 1) * 128],
                        id16_sb[0:H + 1, 0:H + 1],
                    )
                    r_sb = ep.tile([128, 1], F32, tag="recip", name="recip")
                    nc.vector.reciprocal(r_sb[:], tr_ps[:, H:H + 1])
                    o_sb = ep.tile([128, H], F32, tag="osb", name="osb")
                    nc.vector.tensor_scalar_mul(o_sb[:], tr_ps[:, 0:H], r_sb[:])
                    row = slot * CH + j * 128
                    # sync/scalar only: gpsimd stores would hold up its
                    # end-of-kernel queue drain
                    (nc.sync if j % 2 == 0 else nc.scalar).dma_start(
                        out[row:row + 128, :], o_sb[:])

            # ---- slot B: V transposes for h1 tiles, masked-first scores,
            # single-accumulator AV (3-deep score prefetch); slot-A
            # epilogue interleaves ----
            kts = list(range(8, 16)) + list(range(8))
            b_w = {}
            for kt in (8, 9, 10, 11):
                vtr(kt)
            for kt in (8, 9):
                b_w[kt] = wexp(1, kt, score(1, kt))
            for kt in (12, 13, 14, 15):
                vtr(kt)
            b_w[10] = wexp(1, 10, score(1, 10))

            for i, kt in enumerate(kts):
                if kt not in b_w:
                    b_w[kt] = wexp(1, kt, score(1, kt))
                av(av_b, kt, b_w.pop(kt), first=(i == 0), last=(i == NKT - 1))
                nxt = kts[i + 3] if i + 3 < NKT else None
                if nxt is not None and nxt not in b_w:
                    b_w[nxt] = wexp(1, nxt, score(1, nxt))
                if i == 1:
                    epilogue(0, av_a)
            epilogue(1, av_b)

            for pool in (sp2, avpB, avpA, sp):
                pool.release()

    nc.compile()
    return nc


def _host_inputs(x, Wq, bq, Wk, bk, Wv, bv):
    """Build the 8 per-core input maps (all SBUF-layout, fp16/f32)."""
    f16 = np.float16
    Wkv = np.concatenate([Wk, Wv], axis=1)          # [D, 128]
    # wkv[p, kt*128+j] = Wkv[kt*128+p, j]
    wkv_np = np.ascontiguousarray(
        Wkv.reshape(8, 128, 128).transpose(1, 0, 2).reshape(128, 8 * 128)
    ).astype(f16)
    wq_np = np.zeros((128, 8 * H), dtype=f16)
    for kt in range(8):
        wq_np[:, kt * H:(kt + 1) * H] = Wq[kt * 128:(kt + 1) * 128, :]
    bkv_np = np.concatenate([bk, bv]).reshape(128, 1).astype(np.float32)
    bq_np = bq.reshape(H, 1).astype(np.float32)
    qio_np = np.broadcast_to(np.arange(CH, dtype=f16), (128, CH)).copy()
    idv_np = np.concatenate([np.eye(H), np.eye(H)], axis=0).astype(f16)
    id16_np = np.eye(H + 1, dtype=f16)

    in_maps = []
    for c in range(8):
        b = c // 2
        cA, cB = c % 2, 3 - c % 2
        perm = (cA, 1 - cA, 5 - cB, cB)        # chunk order along k
        xTp = np.concatenate(
            [x[b, p * CH:(p + 1) * CH].T for p in perm], axis=1)  # [D, S]
        xTp = xTp.astype(f16)
        # xk[p, (h*8+kt)*1024 + c] = xTp[kt*128+p, h*1024+c]
        xk_np = np.ascontiguousarray(
            xTp.reshape(8, 128, 2, 1024).transpose(1, 2, 0, 3)
            .reshape(128, 16 * 1024))
        # k_global of permuted position p: perm[p//512]*512 + p%512
        pos = np.arange(S)
        kg = np.array(perm)[pos // CH] * CH + pos % CH
        p = np.arange(128)
        thrd_np = np.zeros((128, 8), dtype=np.float32)
        for j in range(4):                      # A diag tiles 0..3
            thrd_np[:, j] = kg[j * 128 + p] - cA * CH
        for j in range(4):                      # B diag tiles 12..15
            thrd_np[:, 4 + j] = kg[(12 + j) * 128 + p] - cB * CH
        thrb_np = np.zeros((128, 8), dtype=np.float32)
        thrb_np[:, 0:4] = -CBIG if (1 - cA) > cA else 0.0   # A tiles 4..7
        thrb_np[:, 4:8] = -CBIG if (5 - cB) > cB else 0.0   # B tiles 8..11
        in_maps.append({
            "xk": xk_np, "wkv": wkv_np, "wq": wq_np,
            "bkv": bkv_np, "bq": bq_np, "qio": qio_np,
            "thrd": thrd_np, "thrb": thrb_np,
            "idv": idv_np, "id16": id16_np,
        })
    return in_maps


def _gather(results, dtype):
    y = np.zeros((B, S, H), dtype=dtype)
    for c in range(8):
        b = c // 2
        cA, cB = c % 2, 3 - c % 2
        o = results[c]["out"]
        y[b, cA * CH:(cA + 1) * CH] = o[:CH]
        y[b, cB * CH:(cB + 1) * CH] = o[CH:]
    return y


def get_nc():
    if "nc" not in _CACHE:
        _CACHE["nc"] = _build_nc()
    return _CACHE["nc"]


def kernel(x, Wq, bq, Wk, bk, Wv, bv, _trace=False, _trace_kwargs=None):
    from concourse.bass_utils import run_bass_kernel_spmd

    x = np.asarray(x, dtype=np.float32)
    Wq, bq = np.asarray(Wq, np.float32), np.asarray(bq, np.float32)
    Wk, bk = np.asarray(Wk, np.float32), np.asarray(bk, np.float32)
    Wv, bv = np.asarray(Wv, np.float32), np.asarray(bv, np.float32)

    nc = get_nc()
    in_maps = _host_inputs(x, Wq, bq, Wk, bk, Wv, bv)
    res = run_bass_kernel_spmd(
        nc, in_maps, core_ids=list(range(8)),
        trace=_trace, **(_trace_kwargs or {}))
    _CACHE["last_result"] = res
    return _gather(res.results, x.dtype)


# revision 24
# speedup vs baseline: 1.1165x; 1.0788x over previous
"""Trainium2 Bass kernel: single-head causal attention, SPMD over 8 NeuronCores.

Problem: x [4, 2048, 1024] f32; Wq/Wk/Wv [1024, 64]; bq/bk/bv [64].
  q,k,v = x@W + b ; out = softmax(causal(q k^T / 8)) @ v  -> [4, 2048, 64]

Sharding (uniform SPMD structure on every core):
  core c -> batch b = c//2 ; query chunks (cA, cB) = (c%2, 3-c%2), 512 rows
  each (pairing an early with a late chunk balances causal work).  Every core
  computes K/V for its batch's full 2048 rows.

Key layout: the k-axis is permuted PER CORE to chunk order
  [cA, 1-cA, 5-cB, cB], so the core's own query columns sit at the STATIC
  positions 0:512 and 1536:2048 of the K/V input.  With that permutation the
  24 (slot, k-tile) score tiles fall into three static classes:
    diag        A:0-3,  B:12-15  -- element-wise causal mask
    conditional A:4-7,  B:8-11   -- fully dead or fully allowed per core
    full        B:0-7            -- causally full for every core
  Masking is folded into the exp: diag tiles add a precomputed 0/+512 bias
  tile then exp(s*scale - 64); conditional tiles add a per-core 0/-400 bias
  column; dead tiles underflow to exactly 0 in fp16, so no mask multiplies
  and the 65th "ones" V row still accumulates the correct denominator.

  Projections produce Q^T/K^T/V^T [64, rows]; scores are computed transposed
  ([k_part, q_free]) so the weight matrix feeds the AV matmul as the moving
  operand; V is re-transposed through 16 small PE transposes.  Both slots
  accumulate AV in a single K=128 PSUM accumulator.  Slot-A attention and
  the V transposes are interleaved into the h1 projection stream so the PE
  never idles (keeps the tensor clock in its high p-state).

  Input DMAs are coalesced: xk is repacked host-side to [128, 16*1024] in
  consumption order, loaded by 8 large descriptors spread over the
  sync/scalar/vector hardware queues; tiny constants ride the gpsimd queue.

dtypes: fp16 SBUF operands, fp32 PSUM accumulation, fp32 epilogue + output.
"""

import os
import sys

import numpy as np

if "/opt/trn_rl_repo" not in sys.path:
    sys.path.insert(0, "/opt/trn_rl_repo")

B, S, D, H = 4, 2048, 1024, 64
CH = 512          # query chunk width
QR = 2 * CH       # query rows per core
NKT = S // 128    # 16 k-tiles of 128
SCALE = 1.0 / np.sqrt(H)
MBIG = 512.0      # diag mask additive bias (exactly representable, *SCALE=64)
CBIG = 400.0      # conditional (dead-tile) bias magnitude

_CACHE = {}


def _build_nc():
    import concourse.bacc as bacc
    import concourse.mybir as mybir
    import concourse.tile as tile

    DT = mybir.dt.float16
    F32 = mybir.dt.float32
    Exp = mybir.ActivationFunctionType.Exp
    Copy = mybir.ActivationFunctionType.Copy
    ge = mybir.AluOpType.is_ge
    mult = mybir.AluOpType.mult
    add = mybir.AluOpType.add

    nc = bacc.Bacc("TRN2", target_bir_lowering=False, debug=False, num_devices=8)

    # xk: k-permuted x^T, [128, 16*1024]; column block h*8+kt holds
    # dmodel-tile kt of k-half h (consumption order, so one coalesced
    # stream fills it front to back).
    xk = nc.dram_tensor("xk", [128, 16 * 1024], DT, kind="ExternalInput")
    wkv = nc.dram_tensor("wkv", [128, 8 * 128], DT, kind="ExternalInput")
    wq = nc.dram_tensor("wq", [128, 8 * H], DT, kind="ExternalInput")
    # constant blobs: cb32 = [bkv | bq | thrd(8) | thrb(8)],
    # cb16 = [qio(512) | idv(64) | id16(65)]
    cb32 = nc.dram_tensor("cb32", [128, 18], F32, kind="ExternalInput")
    cb16 = nc.dram_tensor("cb16", [128, CH + H + H + 1], DT,
                          kind="ExternalInput")
    out = nc.dram_tensor("out", [QR, H], F32, kind="ExternalOutput")

    def xcol(kt, h, sub=0, n=512):
        base = (h * 8 + kt) * 1024 + sub * 512
        return slice(base, base + n)

    with tile.TileContext(nc) as tc:
        with (
            tc.tile_pool(name="const", bufs=1) as cp,
            tc.tile_pool(name="work", bufs=8) as wp,
            tc.tile_pool(name="epi", bufs=4) as ep,
        ):
            # ---- coalesced input DMAs: first pieces split across both hw
            # queues for latency, then big consumption-ordered pieces ----
            wkv_sb = cp.tile([128, 8 * 128], DT, tag="wkv", name="wkv")
            xk_sb = cp.tile([128, 16 * 1024], DT, tag="xk", name="xk")
            wq_sb = cp.tile([128, 8 * H], DT, tag="wq", name="wq")
            cb32_sb = cp.tile([128, 18], F32, tag="cb32", name="cb32")
            cb16_sb = cp.tile([128, CH + 2 * H + 1], DT, tag="cb16",
                              name="cb16")

            nc.sync.dma_start(wkv_sb[0:64, :], wkv[0:64, :])
            nc.scalar.dma_start(wkv_sb[64:128, :], wkv[64:128, :])
            nc.sync.dma_start(xk_sb[0:64, 0:512], xk[0:64, 0:512])
            nc.scalar.dma_start(xk_sb[64:128, 0:512], xk[64:128, 0:512])
            nc.sync.dma_start(xk_sb[0:64, 512:1024], xk[0:64, 512:1024])
            nc.scalar.dma_start(xk_sb[64:128, 512:1024], xk[64:128, 512:1024])
            nc.sync.dma_start(wq_sb[:], wq[:])
            nc.scalar.dma_start(cb32_sb[:], cb32[:])
            nc.sync.dma_start(xk_sb[:, 1024:2048], xk[:, 1024:2048])     # h0 kt1
            nc.scalar.dma_start(xk_sb[:, 4096:6144], xk[:, 4096:6144])   # h0 kt4-5
            nc.sync.dma_start(xk_sb[:, 2048:4096], xk[:, 2048:4096])     # h0 kt2-3
            nc.scalar.dma_start(xk_sb[:, 8192:11264], xk[:, 8192:11264])  # h1 kt0-2
            nc.sync.dma_start(cb16_sb[:], cb16[:])
            nc.sync.dma_start(xk_sb[:, 6144:8192], xk[:, 6144:8192])     # h0 kt6-7
            nc.scalar.dma_start(xk_sb[:, 13312:16384], xk[:, 13312:16384])  # h1 kt5-7
            nc.sync.dma_start(xk_sb[:, 11264:13312], xk[:, 11264:13312])  # h1 kt3-4

            kvT_sb = cp.tile([128, S], DT, tag="kvT", name="kvT")  # 0:64 K^T, 64:128 V^T
            qT_sb = cp.tile([H, QR], DT, tag="qT", name="qT")      # A cols 0:512, B 512:1024
            v_sb = cp.tile([128, NKT * (H + 1)], DT, tag="v", name="v")
            mb_sb = cp.tile([128, 8 * CH], DT, tag="mb", name="mb")  # diag 0/1 mask

            # ---- projections in two 3-bank PSUM phase scopes ----
            def proj_group(h, kt, kv_ps, q_ps, first, last):
                for sub in range(2):
                    nc.tensor.matmul(
                        kv_ps[sub][:],
                        wkv_sb[:, kt * 128:(kt + 1) * 128],
                        xk_sb[:, xcol(kt, h, sub)],
                        start=first, stop=last,
                    )
                # q columns: slot A = sub 0 of h0, slot B = sub 1 of h1
                nc.tensor.matmul(
                    q_ps[:],
                    wq_sb[:, kt * H:(kt + 1) * H],
                    xk_sb[:, xcol(kt, h, sub=h)],
                    start=first, stop=last,
                )

            def proj_epilogue(h, kv_ps, q_ps):
                for sub in range(2):
                    nb = 2 * h + sub
                    nc.vector.tensor_scalar(
                        kvT_sb[:, nb * 512:(nb + 1) * 512], kv_ps[sub][:],
                        cb32_sb[:, 0:1], None, add)
                nc.vector.tensor_scalar(
                    qT_sb[:, h * 512:(h + 1) * 512], q_ps[:],
                    cb32_sb[0:H, 1:2], None, add)

            # ---- phase h0, consumption order matching DMA arrival ----
            H0_ORDER = (0, 1, 4, 5, 2, 3, 6, 7)
            pp0 = tc.alloc_tile_pool(name="proj_ps0", bufs=1, space="PSUM")
            kv_ps0 = [pp0.tile([128, 512], F32, tag=f"kvps0{s}",
                               name=f"kvps0{s}") for s in range(2)]
            q_ps0 = pp0.tile([H, 512], F32, tag="qps0", name="qps0")
            for i, kt in enumerate(H0_ORDER):
                proj_group(0, kt, kv_ps0, q_ps0, first=(i == 0), last=(i == 7))
            proj_epilogue(0, kv_ps0, q_ps0)
            pp0.release()

            sp = tc.alloc_tile_pool(name="score_ps", bufs=4, space="PSUM")
            avpA = tc.alloc_tile_pool(name="avA_ps", bufs=1, space="PSUM")
            av_a = avpA.tile([H + 1, 512], F32, tag="avA", name="avA")

            # diag 0/1 fp16 masks (A tiles 0..3 -> cols 0..3, B tiles
            # 12..15 -> cols 4..7)
            for j in range(8):
                nc.vector.tensor_scalar(
                    mb_sb[:, j * CH:(j + 1) * CH], cb16_sb[:, 0:CH],
                    cb32_sb[:, 2 + j:3 + j], None, ge)

            def vtr(kt):
                t = sp.tile([128, H], DT, tag="score", name=f"vtr{kt}")
                nc.tensor.transpose(
                    t[:], kvT_sb[H:128, kt * 128:(kt + 1) * 128],
                    cb16_sb[H:128, CH:CH + H], tile_position=(H, 0))
                nc.vector.tensor_copy(
                    v_sb[:, kt * (H + 1):kt * (H + 1) + H], t[:])

            nc.vector.memset(v_sb[:], 1.0)
            for kt in range(8):
                vtr(kt)

            # ---- slot helpers ----
            def score(slot, kt):
                s_ps = sp.tile([128, 512], F32, tag="score", name=f"s{slot}{kt}")
                nc.tensor.matmul(
                    s_ps[:],
                    kvT_sb[0:H, kt * 128:(kt + 1) * 128],
                    qT_sb[:, slot * 512:(slot + 1) * 512],
                    start=True, stop=True, tile_position=(0, 0),
                )
                return s_ps

            def wexp(slot, kt, s_ps):
                # tile class: diag (exp then 0/1 mask multiply), cond
                # (per-core bias column folded into exp), full (plain exp)
                w_sb = wp.tile([128, 512], DT, tag="wexp", name="wexp")
                if slot == 0 and kt < 4:
                    j = kt                         # A diag
                elif slot == 1 and kt >= 12:
                    j = kt - 8                     # B diag
                else:
                    j = None
                if j is not None:
                    nc.scalar.activation(w_sb[:], s_ps[:], Exp,
                                         scale=float(SCALE))
                    wm_sb = wp.tile([128, 512], DT, tag="wm", name="wm")
                    nc.vector.tensor_tensor(
                        wm_sb[:], w_sb[:], mb_sb[:, j * CH:(j + 1) * CH],
                        mult)
                    return wm_sb
                if (slot == 0 and kt >= 4) or (slot == 1 and 8 <= kt < 12):
                    nc.scalar.activation(w_sb[:], s_ps[:], Exp,
                                         bias=cb32_sb[:, 6 + kt:7 + kt],
                                         scale=float(SCALE))
                else:                              # B full (kt < 8)
                    nc.scalar.activation(w_sb[:], s_ps[:], Exp,
                                         scale=float(SCALE))
                return w_sb

            def av(acc, kt, w_sb, first, last):
                vs = slice(kt * (H + 1), (kt + 1) * (H + 1))
                nc.tensor.matmul(
                    acc[:], v_sb[:, vs], w_sb[:],
                    start=first, stop=last,
                )

            # ---- phase h1 interleaved with slot A attention ----
            pp1 = tc.alloc_tile_pool(name="proj_ps1", bufs=1, space="PSUM")
            kv_ps1 = [pp1.tile([128, 512], F32, tag=f"kvps1{s}",
                               name=f"kvps1{s}") for s in range(2)]
            q_ps1 = pp1.tile([H, 512], F32, tag="qps1", name="qps1")

            a_s = {}
            a_w = {}

            def asc_pair(p):
                for kt in (2 * p, 2 * p + 1):
                    a_s[kt] = score(0, kt)
                    a_w[kt] = wexp(0, kt, a_s[kt])

            def ava_pair(p):
                for kt in (2 * p, 2 * p + 1):
                    av(av_a, kt, a_w[kt], first=(kt == 0), last=(kt == 7))

            for kt in range(8):
                proj_group(1, kt, kv_ps1, q_ps1, first=(kt == 0),
                           last=(kt == 7))
                # interleave slot-A work behind the projection groups
                if kt == 0:
                    asc_pair(0)
                elif kt == 1:
                    asc_pair(1)
                elif kt == 2:
                    ava_pair(0)
                elif kt == 3:
                    asc_pair(2)
                elif kt == 4:
                    ava_pair(1)
                elif kt == 5:
                    asc_pair(3)
                elif kt == 6:
                    ava_pair(2)
                else:
                    ava_pair(3)
            proj_epilogue(1, kv_ps1, q_ps1)
            pp1.release()

            avpB = tc.alloc_tile_pool(name="avB_ps", bufs=1, space="PSUM")
            av_b = avpB.tile([H + 1, 512], F32, tag="avB", name="avB")
            sp2 = tc.alloc_tile_pool(name="otr_ps", bufs=2, space="PSUM")

            def epilogue(slot, acc):
                oav_sb = ep.tile([H + 1, 512], DT, tag="oav16", name="oav")
                nc.vector.tensor_copy(oav_sb[:], acc[:])
                for j in range(4):
                    tr_ps = sp2.tile([128, H + 1], DT, tag="otr", name="otr")
                    nc.tensor.transpose(
                        tr_ps[:],
                        oav_sb[:, j * 128:(j + 1) * 128],
                        cb16_sb[0:H + 1, CH + H:CH + 2 * H + 1],
                    )
                    r_sb = ep.tile([128, 1], F32, tag="recip", name="recip")
                    nc.vector.reciprocal(r_sb[:], tr_ps[:, H:H + 1])
                    o_sb = ep.tile([128, H], F32, tag="osb", name="osb")
                    nc.vector.tensor_scalar_mul(o_sb[:], tr_ps[:, 0:H], r_sb[:])
                    row = slot * CH + j * 128
                    # sync/scalar only: gpsimd stores would hold up its
                    # end-of-kernel queue drain
                    (nc.sync if j % 2 == 0 else nc.scalar).dma_start(
                        out[row:row + 128, :], o_sb[:])

            # ---- slot B: V transposes for h1 tiles, masked-first scores,
            # single-accumulator AV (3-deep score prefetch); slot-A
            # epilogue interleaves ----
            kts = list(range(8, 16)) + list(range(8))
            b_w = {}
            for kt in (8, 9, 10, 11):
                vtr(kt)
            for kt in (8, 9):
                b_w[kt] = wexp(1, kt, score(1, kt))
            for kt in (12, 13, 14, 15):
                vtr(kt)
            b_w[10] = wexp(1, 10, score(1, 10))

            for i, kt in enumerate(kts):
                if kt not in b_w:
                    b_w[kt] = wexp(1, kt, score(1, kt))
                av(av_b, kt, b_w.pop(kt), first=(i == 0), last=(i == NKT - 1))
                nxt = kts[i + 3] if i + 3 < NKT else None
                if nxt is not None and nxt not in b_w:
                    b_w[nxt] = wexp(1, nxt, score(1, nxt))
                if i == 1:
                    epilogue(0, av_a)
            epilogue(1, av_b)

            for pool in (sp2, avpB, avpA, sp):
                pool.release()

    nc.compile()
    return nc


def _host_inputs(x, Wq, bq, Wk, bk, Wv, bv):
    """Build the 8 per-core input maps (all SBUF-layout, fp16/f32)."""
    f16 = np.float16
    Wkv = np.concatenate([Wk, Wv], axis=1)          # [D, 128]
    # wkv[p, kt*128+j] = Wkv[kt*128+p, j]
    wkv_np = np.ascontiguousarray(
        Wkv.reshape(8, 128, 128).transpose(1, 0, 2).reshape(128, 8 * 128)
    ).astype(f16)
    wq_np = np.zeros((128, 8 * H), dtype=f16)
    for kt in range(8):
        wq_np[:, kt * H:(kt + 1) * H] = Wq[kt * 128:(kt + 1) * 128, :]
    # cb16 = [qio(512) | idv(64) | id16(65)]
    cb16_np = np.zeros((128, CH + 2 * H + 1), dtype=f16)
    cb16_np[:, 0:CH] = np.arange(CH, dtype=f16)
    cb16_np[:, CH:CH + H] = np.concatenate([np.eye(H), np.eye(H)], axis=0)
    cb16_np[0:H + 1, CH + H:] = np.eye(H + 1)

    in_maps = []
    for c in range(8):
        b = c // 2
        cA, cB = c % 2, 3 - c % 2
        perm = (cA, 1 - cA, 5 - cB, cB)        # chunk order along k
        xTp = np.concatenate(
            [x[b, p * CH:(p + 1) * CH].T for p in perm], axis=1)  # [D, S]
        xTp = xTp.astype(f16)
        # xk[p, (h*8+kt)*1024 + c] = xTp[kt*128+p, h*1024+c]
        xk_np = np.ascontiguousarray(
            xTp.reshape(8, 128, 2, 1024).transpose(1, 2, 0, 3)
            .reshape(128, 16 * 1024))
        # k_global of permuted position p: perm[p//512]*512 + p%512
        pos = np.arange(S)
        kg = np.array(perm)[pos // CH] * CH + pos % CH
        p = np.arange(128)
        # cb32 = [bkv | bq(pad) | thrd(8) | thrb(8, pre-scaled)]
        cb32_np = np.zeros((128, 18), dtype=np.float32)
        cb32_np[:, 0] = np.concatenate([bk, bv])
        cb32_np[0:H, 1] = bq
        for j in range(4):                      # A diag tiles 0..3
            cb32_np[:, 2 + j] = kg[j * 128 + p] - cA * CH
        for j in range(4):                      # B diag tiles 12..15
            cb32_np[:, 6 + j] = kg[(12 + j) * 128 + p] - cB * CH
        cb32_np[:, 10:14] = (-CBIG * SCALE) if (1 - cA) > cA else 0.0
        cb32_np[:, 14:18] = (-CBIG * SCALE) if (5 - cB) > cB else 0.0
        in_maps.append({
            "xk": xk_np, "wkv": wkv_np, "wq": wq_np,
            "cb32": cb32_np, "cb16": cb16_np,
        })
    return in_maps


def _gather(results, dtype):
    y = np.zeros((B, S, H), dtype=dtype)
    for c in range(8):
        b = c // 2
        cA, cB = c % 2, 3 - c % 2
        o = results[c]["out"]
        y[b, cA * CH:(cA + 1) * CH] = o[:CH]
        y[b, cB * CH:(cB + 1) * CH] = o[CH:]
    return y


def get_nc():
    if "nc" not in _CACHE:
        _CACHE["nc"] = _build_nc()
    return _CACHE["nc"]


def kernel(x, Wq, bq, Wk, bk, Wv, bv, _trace=False, _trace_kwargs=None):
    from concourse.bass_utils import run_bass_kernel_spmd

    x = np.asarray(x, dtype=np.float32)
    Wq, bq = np.asarray(Wq, np.float32), np.asarray(bq, np.float32)
    Wk, bk = np.asarray(Wk, np.float32), np.asarray(bk, np.float32)
    Wv, bv = np.asarray(Wv, np.float32), np.asarray(bv, np.float32)

    nc = get_nc()
    in_maps = _host_inputs(x, Wq, bq, Wk, bk, Wv, bv)
    res = run_bass_kernel_spmd(
        nc, in_maps, core_ids=list(range(8)),
        trace=_trace, **(_trace_kwargs or {}))
    _CACHE["last_result"] = res
    return _gather(res.results, x.dtype)
